# revision 15
# baseline (speedup 1.0000x reference)
"""Chamfer distance (pytorch3d-style, with normals) on 8 Trainium2 cores.

Problem: B=4, N=M=8192, D=3.
  d[b,n,m] = |x_n|^2 + |y_m|^2 - 2 x_n.y_m
  cham_dist    = mean_n min_m d + mean_m min_n d
  cham_normals = mean(1-|cos(nx_n, ny_argmin)|) + mean(1-|cos(ny_m, nx_argmin)|)

Sharding: 8 independent units = 2 sides x 4 batches, one per core.
Each core brute-forces one full 8192x8192 row-max problem:
  h[n,m] = 2 x_n.y_m - |y_m|^2   (K=4 matmul: rows [2x0,2x1,2x2,1] x [y0,y1,y2,-|y|^2])
  min_m d[n,m] = |x_n|^2 - max_m h[n,m];  argmin d = argmax h.
Device outputs per core: per-row max of h and its argmax index.
Host does the (tiny) remainder: means, normal gather, cosine similarity.
"""

import sys

import numpy as np

if "/opt/trn_rl_repo" not in sys.path:  # harmless if already importable
    sys.path.insert(0, "/opt/trn_rl_repo")

B, N, M, D = 4, 8192, 8192, 3
P = 128          # rows per block (SBUF partitions)
NBLK = N // P    # 64 row-blocks
FD = 512         # matmul free dim = one PSUM bank
QCOLS = 2048     # columns per PSUM tile (4 banks); 4 quarter-tiles per row-block
NQ = M // QCOLS  # 4
EPS = 1e-6

_cache = {}


def _build_bass():
    import concourse.bacc as bacc
    import concourse.mybir as mybir
    from concourse import tile

    f32 = mybir.dt.float32
    u32 = mybir.dt.uint32

    nc = bacc.Bacc("TRN2", target_bir_lowering=False, debug=False)
    # ab[:, :N]  rows: [2*x0, 2*x1, 2*x2, ones]  over n
    # ab[:, N:]  rows: [y0, y1, y2, -|y|^2]      over m
    # one tensor + one DMA so matmuls depend on a single semaphore
    ab = nc.declare_dram_parameter("ab", [4, N + M], f32, isOutput=False)
    out_val = nc.declare_dram_parameter("val", [P, NBLK], f32, isOutput=True)
    out_idx = nc.declare_dram_parameter("idx", [P, NBLK], u32, isOutput=True)

    with tile.TileContext(nc) as tc:
        with (
            tc.tile_pool(name="inp", bufs=1) as inp,
            tc.tile_pool(name="g", bufs=2) as gp,
            tc.tile_pool(name="ps", bufs=2, space="PSUM") as pp,
            tc.tile_pool(name="small", bufs=4) as sp,
            tc.tile_pool(name="acc", bufs=1) as accp,
        ):
            ab_sb = inp.tile([4, N + M], f32, tag="ab")
            nc.sync.dma_start(ab_sb[:], ab[:])

            val_acc = accp.tile([P, NBLK], f32, tag="vacc")
            idx_acc = accp.tile([P, NBLK], u32, tag="iacc")

            for i in range(NBLK):
                lhsT = ab_sb[:, i * P:(i + 1) * P]
                g = gp.tile([P, M], f32, tag="g")
                for q in range(NQ):
                    ps = pp.tile([P, QCOLS], f32, tag="ps")
                    for t in range(QCOLS // FD):
                        c0 = q * QCOLS + t * FD
                        nc.tensor.matmul(
                            ps[:, t * FD:(t + 1) * FD],
                            lhsT,
                            ab_sb[:, N + c0:N + c0 + FD],
                            start=True,
                            stop=True,
                        )
                    nc.scalar.activation(
                        g[:, q * QCOLS:(q + 1) * QCOLS], ps[:],
                        mybir.ActivationFunctionType.Copy,
                    )
                top8 = sp.tile([P, 8], f32, tag="top8")
                idx8 = sp.tile([P, 8], u32, tag="idx8")
                nc.vector.max(top8[:], g[:])
                nc.vector.max_index(idx8[:], top8[:], g[:])
                # gpsimd casts through f32 and corrupts u32 — keep on DVE
                nc.vector.tensor_copy(val_acc[:, i:i + 1], top8[:, 0:1])
                nc.vector.tensor_copy(idx_acc[:, i:i + 1], idx8[:, 0:1])

            nc.sync.dma_start(out_val[:], val_acc[:])
            nc.sync.dma_start(out_idx[:], idx_acc[:])

    _strip_redundant_matmul_waits(nc)
    nc.compile()
    return nc


_ENGINE_SEM_PREFIX = {
    "PE": "PE",
    "Activation": "Activation",
    "DVE": "DVE",
    "Pool": "Pool",
    "SP": "SP",
}


def _strip_redundant_matmul_waits(nc):
    """Walrus encodes a limited number of sync waits per instruction (1 for
    fp32 self-loading matmuls, 2 for ACT).  Tile's wait emission is not
    transitively minimal: it emits a same-engine wait (pipeline-drain WAW on a
    recycled PSUM/SBUF slot) alongside a cross-engine wait on the consumer
    that read that slot -- and the consumer itself already waited on those
    same-engine increments.  Drop same-engine waits whenever a cross-engine
    wait remains; in this program the cross-engine wait always transitively
    covers the dropped one.
    """
    for f in nc.m.functions:
        for blk in f.blocks:
            for inst in blk.instructions:
                kind = inst.__class__.__name__
                if kind in ("InstDrain", "InstEventSemaphore"):
                    continue
                si = inst.sync_info
                if si is None or len(si.on_wait) <= 1:
                    continue
                eng = str(inst.engine).split(".")[-1]
                pref = _ENGINE_SEM_PREFIX.get(eng, eng)
                keep = [
                    w for w in si.on_wait
                    if not w.ant_name.startswith(pref + "_")
                ]
                if not keep or len(keep) == len(si.on_wait):
                    continue
                if kind == "InstMatmult":
                    assert len(keep) == 1, (
                        f"{inst.name}: {len(keep)} cross-engine waits; cannot "
                        f"encode on a self-loading fp32 matmul: {si.on_wait}"
                    )
                si.on_wait = keep
                inst.sync_info = si


def _get_nc():
    if "nc" not in _cache:
        _cache["nc"] = _build_bass()
    return _cache["nc"]


def _cos_abs(a, b):
    # |cosine similarity| along last axis, pytorch3d-style clamping, fp32
    na = np.maximum(np.linalg.norm(a, axis=-1), EPS).astype(np.float32)
    nb = np.maximum(np.linalg.norm(b, axis=-1), EPS).astype(np.float32)
    return np.abs(np.sum(a * b, axis=-1) / (na * nb))


def _run_spmd(nc, in_maps):
    """Run the 8-core SPMD program; cache the jitted executable across calls.

    Mirrors bass2jax.run_bass_via_pjrt's multi-core branch but hoists the
    shard_map jit out of the per-call path (run_bass_kernel_spmd rebuilds it
    every call, costing ~300ms of retracing).  Falls back to the stock path
    on any mismatch with bass2jax internals.
    """
    try:
        import jax
        import concourse.mybir as mybir
        from concourse import bass2jax
        from jax.experimental.shard_map import shard_map
        from jax.sharding import Mesh, PartitionSpec

        if "runner" not in _cache:
            bass2jax.install_neuronx_cc_hook()
            in_names, out_names, out_avals, zero_outs = [], [], [], []
            part_name = (
                nc.partition_id_tensor.name if nc.partition_id_tensor else None
            )
            for alloc in nc.m.functions[0].allocations:
                if not isinstance(alloc, mybir.MemoryLocationSet):
                    continue
                name = alloc.memorylocations[0].name
                if alloc.kind == "ExternalInput":
                    if name != part_name:
                        in_names.append(name)
                elif alloc.kind == "ExternalOutput":
                    shape = tuple(alloc.tensor_shape)
                    dtype = mybir.dt.np(alloc.dtype)
                    out_names.append(name)
                    out_avals.append(jax.core.ShapedArray(shape, dtype))
                    zero_outs.append(np.zeros(shape, dtype))
            assert nc.dbg_addr is None
            n_params = len(in_names)
            all_names = in_names + out_names
            if part_name is not None:
                all_names = all_names + [part_name]
            all_names = tuple(all_names)

            def _body(*args):
                operands = list(args)
                if part_name is not None:
                    operands.append(bass2jax.partition_id_tensor())
                return tuple(bass2jax._bass_exec_p.bind(
                    *operands,
                    out_avals=tuple(out_avals),
                    in_names=all_names,
                    out_names=tuple(out_names),
                    lowering_input_output_aliases=(),
                    sim_require_finite=True,
                    sim_require_nnan=True,
                    nc=nc,
                ))

            devices = jax.devices()[:8]
            mesh = Mesh(np.asarray(devices), ("core",))
            nio = n_params + len(out_names)
            sharded = jax.jit(
                shard_map(
                    _body, mesh=mesh,
                    in_specs=(PartitionSpec("core"),) * nio,
                    out_specs=(PartitionSpec("core"),) * len(out_names),
                    check_rep=False,
                ),
                donate_argnums=tuple(range(n_params, nio)),
                keep_unused=True,
            )
            _cache["runner"] = (sharded, in_names, out_names, out_avals, zero_outs)

        sharded, in_names, out_names, out_avals, zero_outs = _cache["runner"]
        concat_in = [
            np.concatenate([m[nm] for m in in_maps], axis=0) for nm in in_names
        ]
        concat_zeros = [
            np.zeros((8 * z.shape[0], *z.shape[1:]), z.dtype) for z in zero_outs
        ]
        outs = sharded(*concat_in, *concat_zeros)
        return [
            {nm: np.asarray(outs[i]).reshape(8, *out_avals[i].shape)[c]
             for i, nm in enumerate(out_names)}
            for c in range(8)
        ]
    except Exception:
        _cache.pop("runner", None)
        from concourse.bass_utils import run_bass_kernel_spmd
        return run_bass_kernel_spmd(nc, in_maps, list(range(8))).results


def kernel(xyz1, xyz2, nxyz1, nxyz2):
    xyz1 = np.asarray(xyz1, dtype=np.float32)
    xyz2 = np.asarray(xyz2, dtype=np.float32)
    nxyz1 = np.asarray(nxyz1, dtype=np.float32)
    nxyz2 = np.asarray(nxyz2, dtype=np.float32)

    nc = _get_nc()

    in_maps = []
    xs = []
    for c in range(8):
        s, b = divmod(c, 4)
        x = xyz1[b] if s == 0 else xyz2[b]
        y = xyz2[b] if s == 0 else xyz1[b]
        xs.append(x)
        ab = np.empty((4, N + M), dtype=np.float32)
        ab[0:3, :N] = (2.0 * x).T
        ab[3, :N] = 1.0
        ab[0:3, N:] = y.T
        ab[3, N:] = -np.sum(y * y, axis=-1)
        in_maps.append({"ab": ab})

    results = _run_spmd(nc, in_maps)
    _cache["last_results"] = results

    cham = np.zeros(2, dtype=np.float64)
    chamn = np.zeros(2, dtype=np.float64)
    for c in range(8):
        s, b = divmod(c, 4)
        val = np.asarray(results[c]["val"])  # [P, NBLK] f32
        idx = np.asarray(results[c]["idx"])  # [P, NBLK] u32
        hmax = val.T.reshape(-1)                 # n-ordered [8192]
        am = idx.T.reshape(-1).astype(np.int64)  # argmin over the other cloud
        x = xs[c]
        x2 = np.sum(x * x, axis=-1)              # f32
        mind = x2 - hmax                         # f32 [8192]
        cham[s] += float(np.mean(mind, dtype=np.float64))

        own_normals = nxyz1[b] if s == 0 else nxyz2[b]
        other_normals = nxyz2[b] if s == 0 else nxyz1[b]
        gathered = other_normals[am]             # [8192, 3]
        cn = 1.0 - _cos_abs(own_normals, gathered)
        chamn[s] += float(np.mean(cn, dtype=np.float64))

    cham_dist = np.float32(cham[0] / B + cham[1] / B)
    cham_normals = np.float32(chamn[0] / B + chamn[1] / B)
    return cham_dist, cham_normals
